# revision 6
# baseline (speedup 1.0000x reference)
"""Trainium2 Bass kernel for the C-LIF spiking-neuron forward pass.

Problem: x [16, 8192, 200] fp32, scalar decays dm=0.9, ds=0.6, VTH=0.5.
Per neuron, over time t:
    M = dm*(M + x_t); S = ds*(S + x_t); E = dm*E + o_prev*VTH
    u = M - S - E;    o_t = (u - VTH > 0)

Reformulation used on-chip (all fp32, bit-faithful to the reference up to
benign reassociation):
  * A_t := 2*(M_t - S_t) is a 2-pole linear filter of x whose transfer
    function has a CONSTANT numerator: A = cascade of two one-pole IIRs
    applied to x' = 2*(dm-ds)*x.  Each one-pole IIR is exactly one DVE
    `tensor_tensor_scan` instruction per [128, 200] tile.
  * With F := E/VTH (VTH=0.5 is a power of two, so the scaling is exact):
        F_t = dm*F_{t-1} + o_{t-1}
        o_t = ((F_t + 1) < A_t)
    i.e. exactly two fused `scalar_tensor_tensor` DVE ops per time step
    over all 16384 neurons a core owns.

Sharding: 131072 neuron rows split evenly across 8 cores (data parallel,
no cross-core communication).  Spikes are produced as bf16 (exact for
0/1) to halve the output DMA traffic; the host upcasts to fp32.
"""

import numpy as np

# ---------------------------------------------------------------- constants
B, N, T = 16, 8192, 200
DM, DS, VTH = 0.9, 0.6, 0.5
N_CORES = 8
ROWS = B * N                      # 131072 neuron rows
ROWS_PER_CORE = ROWS // N_CORES   # 16384
G = ROWS_PER_CORE // 128          # 128 groups of 128 neurons
GB = 8                            # groups per DMA batch
NB = G // GB                      # 16 batches

_cached = {}


def _build_program(iters: int = 1):
    import concourse.mybir as mybir
    from concourse import bacc, tile
    from contextlib import nullcontext

    fp32 = mybir.dt.float32
    bf16 = mybir.dt.bfloat16
    Alu = mybir.AluOpType

    nc = bacc.Bacc("TRN2", target_bir_lowering=False, debug=False)
    x_d = nc.dram_tensor("x", [ROWS_PER_CORE, T], fp32, kind="ExternalInput").ap()
    o_d = nc.dram_tensor("o", [ROWS_PER_CORE, T], bf16, kind="ExternalOutput").ap()

    with tile.TileContext(nc) as tc:
        with (
            tc.tile_pool(name="xb", bufs=2) as xb_pool,
            tc.tile_pool(name="xp", bufs=2) as xp_pool,
            tc.tile_pool(name="y1", bufs=2) as y1_pool,
            tc.tile_pool(name="big", bufs=1) as big_pool,
            tc.tile_pool(name="consts", bufs=1) as const_pool,
        ):
            abig = big_pool.tile([128, T * G], fp32)     # [p, (t g)] time-major
            onat = big_pool.tile([128, G * T], bf16)     # [p, (g t)] neuron-major
            dmt = const_pool.tile([128, T], fp32)
            dst = const_pool.tile([128, T], fp32)
            F = const_pool.tile([128, G], fp32)

            loop_cm = tc.For_i(0, iters, 1) if iters > 1 else nullcontext()
            with loop_cm:
                nc.vector.memset(dmt[:], DM)
                nc.vector.memset(dst[:], DS)
                nc.vector.memset(F[:], 0.0)

                a3 = abig[:].rearrange("p (t g) -> p t g", g=G)
                o3 = onat[:].rearrange("p (g t) -> p g t", t=T)

                # -- phase B: load + prescale + two chained IIR scans/group
                for gb in range(NB):
                    xb = xb_pool.tile([128, GB * T], fp32)
                    src = x_d[gb * GB * 128:(gb + 1) * GB * 128, :]
                    nc.sync.dma_start(
                        xb[:].rearrange("p (gs t) -> p gs t", t=T),
                        src.rearrange("(gs p) t -> p gs t", p=128))
                    xp = xp_pool.tile([128, GB * T], fp32)
                    nc.scalar.mul(xp[:], xb[:], 2.0 * (DM - DS))
                    for gs in range(GB):
                        g = gb * GB + gs
                        y1 = y1_pool.tile([128, T], fp32)
                        nc.vector.tensor_tensor_scan(
                            y1[:], dmt[:], xp[:, gs * T:(gs + 1) * T],
                            0.0, Alu.mult, Alu.add)
                        nc.vector.tensor_tensor_scan(
                            a3[:, :, g], dst[:], y1[:],
                            0.0, Alu.mult, Alu.add)

                # -- phase C: sequential spike recurrence, 2 DVE ops/step
                nc.vector.scalar_tensor_tensor(
                    o3[:, :, 0], F[:], 1.0, a3[:, 0, :], Alu.add, Alu.is_lt)
                for t in range(1, T):
                    nc.vector.scalar_tensor_tensor(
                        F[:], F[:], DM, o3[:, :, t - 1], Alu.mult, Alu.add)
                    nc.vector.scalar_tensor_tensor(
                        o3[:, :, t], F[:], 1.0, a3[:, t, :], Alu.add, Alu.is_lt)

                # -- phase D: store spikes (bf16, contiguous per group)
                for gb in range(NB):
                    dst_ap = o_d[gb * GB * 128:(gb + 1) * GB * 128, :]
                    nc.sync.dma_start(
                        dst_ap.rearrange("(gs p) t -> p gs t", p=128),
                        onat[:, gb * GB * T:(gb + 1) * GB * T]
                            .rearrange("p (gs t) -> p gs t", t=T))

    nc.compile()
    return nc


def _run(x_flat: np.ndarray, iters: int = 1, trace: bool = False):
    from concourse.bass_utils import run_bass_kernel_spmd

    key = f"nc{iters}"
    if key not in _cached:
        _cached[key] = _build_program(iters)
    nc = _cached[key]
    shards = [
        np.ascontiguousarray(x_flat[c * ROWS_PER_CORE:(c + 1) * ROWS_PER_CORE])
        for c in range(N_CORES)
    ]
    in_maps = [{"x": s} for s in shards]
    res = run_bass_kernel_spmd(nc, in_maps, list(range(N_CORES)), trace=trace)
    outs = [np.asarray(r["o"], dtype=np.float32) for r in res.results]
    return np.concatenate(outs, axis=0), res


def kernel(x, decay_m=None, decay_s=None):
    x = np.asarray(x, dtype=np.float32)
    out_flat, _ = _run(x.reshape(ROWS, T))
    return out_flat.reshape(B, N, T)


# revision 8
# speedup vs baseline: 16.9749x; 16.9749x over previous
"""Trainium2 Bass kernel for the C-LIF spiking-neuron forward pass.

Problem: x [16, 8192, 200] fp32, scalar decays dm=0.9, ds=0.6, VTH=0.5.
Per neuron, over time t:
    M = dm*(M + x_t); S = ds*(S + x_t); E = dm*E + o_prev*VTH
    u = M - S - E;    o_t = (u - VTH > 0)

On-chip reformulation (all fp32, bit-faithful up to benign reassociation):
  * A_t := 2*(M_t - S_t) is a 2-pole linear filter of x whose transfer
    function has a CONSTANT numerator, so A = cascade of two one-pole IIRs
    applied to x' = 2*(dm-ds)*x.  One DVE `tensor_tensor_scan` per pole per
    [128, 200] tile (decay operand as a [128,1] broadcast).  The scan
    results are transposed into a time-major A buffer by ScalarE/GPSIMD
    copies so the sequential phase reads dense tiles.
  * With F := E/VTH (VTH=0.5 is a power of two, so scaling is exact):
        F_t = dm*F_{t-1} + o_{t-1},   o_t = ((F_t + 1) < A_t)
    A runtime-registered custom DVE op fuses one step into ONE instruction:
        F_t = dm*F_{t-1} + ((F_{t-1} + 1) < A_{t-1})
    Spikes are reconstructed afterwards in wide chunks with the identical
    IEEE compare (bit-equal to the in-loop one): o_t = (F_t + 1) < A_t,
    then GPSIMD copies transpose them into a DMA-friendly neuron-major
    bf16 buffer (bf16 is exact for 0/1).  F and o live in ping-pong rings.

Sharding: 131072 neuron rows split evenly across 8 cores (data parallel,
no cross-device communication).  Host upcasts bf16 spikes to fp32.
"""

import numpy as np

# ---------------------------------------------------------------- constants
B, N, T = 16, 8192, 200
DM, DS, VTH = 0.9, 0.6, 0.5
N_CORES = 8
ROWS = B * N                      # 131072 neuron rows
ROWS_PER_CORE = ROWS // N_CORES   # 16384
G = ROWS_PER_CORE // 128          # 128 groups of 128 neurons
GB = 4                            # groups per DMA batch
NB = G // GB                      # 32 batches
TCH = 20                          # time-chunk (columns per recon chunk)
NCH = T // TCH                    # 10 chunks
RING = 2 * TCH                    # ping-pong rings (F fp32, o bf16)

_cached = {}


def _register_lif_op():
    """Runtime-register the fused LIF step op:
    out = in0*s0 + ((in0 + 1) < in1)."""
    from concourse import dve_ops
    from concourse.dve_spec import Spec, Src0, Src1, C0, One, lower
    from concourse.dve_uop import DveOpSpec

    name = "LIF_STEP_ANT"
    for op in dve_ops.OPS:
        if op.name == name:
            return op
    spec = Spec(
        body=Src0 * C0 + ((Src0 + One) < Src1),
        reference=lambda in0, in1, s0, s1, imm2: in0 * s0
        + ((in0 + np.float32(1.0)) < in1).astype(np.float32),
    )
    row = dve_ops._CUSTOM_DVE_ROW_BASE + len(dve_ops.OPS)
    dve_ops._SUB_OPCODE_FOR_NAME[name] = row
    shas = {
        ver: DveOpSpec(name=name, opcode=row, uops=lower(spec, ver=ver),
                       rd1_en=True).sha(ver)
        for ver in ("v3", "v4")
    }
    op = dve_ops.DveOp(name, spec, subdim=False, uops_sha=shas)
    dve_ops.OPS.append(op)
    return op


def _build_program(iters: int = 1):
    import concourse.mybir as mybir
    from concourse import bacc, tile
    from contextlib import nullcontext

    fp32 = mybir.dt.float32
    bf16 = mybir.dt.bfloat16
    Alu = mybir.AluOpType
    lif_op = _register_lif_op()

    nc = bacc.Bacc("TRN2", target_bir_lowering=False, debug=False)
    x_d = nc.dram_tensor("x", [ROWS_PER_CORE, T], fp32, kind="ExternalInput").ap()
    o_d = nc.dram_tensor("o", [ROWS_PER_CORE, T], bf16, kind="ExternalOutput").ap()

    with tile.TileContext(nc) as tc:
        with (
            tc.tile_pool(name="xb", bufs=2) as xb_pool,
            tc.tile_pool(name="ys", bufs=2) as y_pool,
            tc.tile_pool(name="big", bufs=1) as big_pool,
            tc.tile_pool(name="consts", bufs=1) as const_pool,
        ):
            abig = big_pool.tile([128, T * G], fp32)       # [p,(t g)] A'
            onat = big_pool.tile([128, G * T], bf16)       # [p,(g t)] spikes
            frng = big_pool.tile([128, RING * G], fp32)    # F ping-pong ring
            orng = big_pool.tile([128, RING * G], bf16)    # o ping-pong ring
            dm1 = const_pool.tile([128, 1], fp32)
            ds1 = const_pool.tile([128, 1], fp32)

            a3 = abig[:].rearrange("p (t g) -> p t g", g=G)
            f3 = frng[:].rearrange("p (t g) -> p t g", g=G)
            or3 = orng[:].rearrange("p (t g) -> p t g", g=G)
            o3t = onat[:].rearrange("p (g t) -> p t g", t=T)

            loop_cm = tc.For_i(0, iters, 1) if iters > 1 else nullcontext()
            with loop_cm:
                nc.vector.memset(dm1[:], DM)
                nc.vector.memset(ds1[:], DS)
                nc.vector.memset(f3[:, 0, :], 0.0)         # F_0 = 0

                # -- phase B: load, prescale in place, cascade scans,
                #    transpose-copy into time-major A' (ScalarE/GPSIMD split)
                for gb in range(NB):
                    xb = xb_pool.tile([128, GB * T], fp32)
                    src = x_d[gb * GB * 128:(gb + 1) * GB * 128, :]
                    nc.sync.dma_start(
                        xb[:].rearrange("p (gs t) -> p gs t", t=T),
                        src.rearrange("(gs p) t -> p gs t", p=128))
                    nc.scalar.mul(xb[:], xb[:], 2.0 * (DM - DS))
                    for gs in range(GB):
                        g = gb * GB + gs
                        y1 = y_pool.tile([128, T], fp32)
                        nc.vector.tensor_tensor_scan(
                            y1[:], dm1[:].broadcast_to([128, T]),
                            xb[:, gs * T:(gs + 1) * T],
                            0.0, Alu.mult, Alu.add)
                        nc.vector.tensor_tensor_scan(
                            y1[:], ds1[:].broadcast_to([128, T]), y1[:],
                            0.0, Alu.mult, Alu.add)
                        if g % 2 == 0:
                            nc.scalar.copy(a3[:, :, g], y1[:])
                        else:
                            nc.gpsimd.tensor_copy(a3[:, :, g], y1[:])

                # -- phase C: fused F recurrence (1 DVE op / step) + chunked
                #    spike recon (dense DVE) + GPSIMD transpose into onat
                for c in range(NCH):
                    lo, hi = c * TCH, (c + 1) * TCH
                    half = (c % 2) * TCH
                    for t in range(max(1, lo), hi):
                        nc.vector._custom_dve(
                            lif_op,
                            out=f3[:, t % RING, :],
                            in0=f3[:, (t - 1) % RING, :],
                            in1=a3[:, t - 1, :],
                            s0=DM)
                    nc.vector.scalar_tensor_tensor(
                        or3[:, half:half + TCH, :],
                        f3[:, half:half + TCH, :], 1.0,
                        a3[:, lo:hi, :], Alu.add, Alu.is_lt)
                    nc.gpsimd.tensor_copy(
                        o3t[:, lo:hi, :], or3[:, half:half + TCH, :])

                # -- phase D: store spikes (bf16, contiguous per group row)
                for gb in range(NB):
                    dst_ap = o_d[gb * GB * 128:(gb + 1) * GB * 128, :]
                    nc.sync.dma_start(
                        dst_ap.rearrange("(gs p) t -> p gs t", p=128),
                        onat[:, gb * GB * T:(gb + 1) * GB * T]
                            .rearrange("p (gs t) -> p gs t", t=T))

    nc.compile()
    return nc


def _run(x_flat: np.ndarray, iters: int = 1, trace: bool = False):
    from concourse.bass_utils import run_bass_kernel_spmd

    key = f"nc{iters}"
    if key not in _cached:
        _cached[key] = _build_program(iters)
    nc = _cached[key]
    shards = [
        np.ascontiguousarray(x_flat[c * ROWS_PER_CORE:(c + 1) * ROWS_PER_CORE])
        for c in range(N_CORES)
    ]
    in_maps = [{"x": s} for s in shards]
    res = run_bass_kernel_spmd(nc, in_maps, list(range(N_CORES)), trace=trace)
    outs = [np.asarray(r["o"], dtype=np.float32) for r in res.results]
    return np.concatenate(outs, axis=0), res


def kernel(x, decay_m=None, decay_s=None):
    x = np.asarray(x, dtype=np.float32)
    out_flat, _ = _run(x.reshape(ROWS, T))
    return out_flat.reshape(B, N, T)


# revision 12
# speedup vs baseline: 27.8595x; 1.6412x over previous
"""Trainium2 Bass kernel for the C-LIF spiking-neuron forward pass.

Problem: x [16, 8192, 200] fp32, scalar decays dm=0.9, ds=0.6, VTH=0.5.
Per neuron, over time t:
    M = dm*(M + x_t); S = ds*(S + x_t); E = dm*E + o_prev*VTH
    u = M - S - E;    o_t = (u - VTH > 0)

On-chip reformulation (fp32, numerically faithful to the reference up to
~1-ulp reassociation at the spike threshold):
  * 2*(M_t - S_t) = 0.6 * y2_t where y2 = cascade of two one-pole IIRs on
    RAW x (the transfer function of M-S has a constant numerator).  Each
    pole is one DVE `tensor_tensor_scan` per [128, 200] tile; the gain is
    folded into the consumers, so phase B is DMA -> DVE only, with
    ScalarE/GPSIMD doing pure transpose-copies into a time-major buffer.
  * With F := E/VTH (VTH=0.5 is a power of two => exact scaling):
        F_t = dm*F_{t-1} + o_{t-1},   o_t = ((F_t + 1) < 0.6*y2_t)
    Two runtime-registered custom DVE ops:
      step:  F_t = dm*F_{t-1} + ((F_{t-1} + 1) < 0.6*Src1)   (1 op/step)
      recon: o_t = (F_t + 1) < 0.6*Src1                      (wide chunks)
    The compares are the same fused fp32 ALU expression, so the spikes
    the state update saw and the emitted spikes are bit-identical.
    F (fp32) and o (bf16; exact for 0/1) live in 4-deep column rings;
    ScalarE+GPSIMD transpose each o chunk into the neuron-major bf16
    output buffer, which leaves in ONE big DMA.

Sharding: 131072 neuron rows split evenly across 8 cores (data parallel,
no cross-device communication).  Host upcasts bf16 spikes to fp32.
"""

import numpy as np

# ---------------------------------------------------------------- constants
B, N, T = 16, 8192, 200
DM, DS, VTH = 0.9, 0.6, 0.5
GAIN = 2.0 * (DM - DS)            # 0.6: folded into the compares
N_CORES = 8
ROWS = B * N                      # 131072 neuron rows
ROWS_PER_CORE = ROWS // N_CORES   # 16384
G = ROWS_PER_CORE // 128          # 128 groups of 128 neurons
GB = 4                            # groups per DMA batch
NB = G // GB                      # 32 batches
TCH = 10                          # time-chunk (columns per recon chunk)
NCH = T // TCH                    # 20 chunks
NHALF = 4                         # ring depth in chunks
RING = NHALF * TCH                # 40 columns

_cached = {}


def _register_ops():
    """Runtime-register the two fused LIF ops."""
    from concourse import dve_ops
    from concourse.dve_spec import Spec, Src0, Src1, C0, C2, One, lower
    from concourse.dve_uop import DveOpSpec

    def reg(name, spec):
        for op in dve_ops.OPS:
            if op.name == name:
                return op
        row = dve_ops._CUSTOM_DVE_ROW_BASE + len(dve_ops.OPS)
        dve_ops._SUB_OPCODE_FOR_NAME[name] = row
        shas = {
            ver: DveOpSpec(name=name, opcode=row, uops=lower(spec, ver=ver),
                           rd1_en=True).sha(ver)
            for ver in ("v3", "v4")
        }
        op = dve_ops.DveOp(name, spec, subdim=False, uops_sha=shas)
        dve_ops.OPS.append(op)
        return op

    step = reg("LIF_STEP2_ANT", Spec(
        body=Src0 * C0 + ((Src0 + One) < Src1 * C2),
        reference=lambda in0, in1, s0, s1, imm2: in0 * s0
        + ((in0 + np.float32(1.0)) < in1 * np.float32(imm2)).astype(np.float32),
    ))
    recon = reg("LIF_RECON_ANT", Spec(
        body=(Src0 + One) < Src1 * C2,
        reference=lambda in0, in1, s0, s1, imm2:
        ((in0 + np.float32(1.0)) < in1 * np.float32(imm2)).astype(np.float32),
    ))
    return step, recon


def _build_program(iters: int = 1, phases: str = "full", timing: bool = False):
    import concourse.mybir as mybir
    from concourse import bacc, tile
    from contextlib import nullcontext

    fp32 = mybir.dt.float32
    bf16 = mybir.dt.bfloat16
    Alu = mybir.AluOpType
    step_op, recon_op = _register_ops()

    nc = bacc.Bacc("TRN2", target_bir_lowering=False, debug=False)
    if timing:
        # tiny external I/O + internal DRAM scratch: same on-device work,
        # no host<->device transfer noise in wall-clock measurements
        nc.dram_tensor("x", [128, T], fp32, kind="ExternalInput")
        o_ext = nc.dram_tensor("o", [128, T], bf16, kind="ExternalOutput").ap()
        x_d = nc.dram_tensor("xs", [ROWS_PER_CORE, T], fp32).ap()
        o_d = nc.dram_tensor("os", [ROWS_PER_CORE, T], bf16).ap()
    else:
        x_d = nc.dram_tensor("x", [ROWS_PER_CORE, T], fp32,
                             kind="ExternalInput").ap()
        o_d = nc.dram_tensor("o", [ROWS_PER_CORE, T], bf16,
                             kind="ExternalOutput").ap()

    do_b = "B" in phases or phases == "full"
    do_c = "C" in phases or phases == "full"
    do_d = "D" in phases or phases == "full"

    with tile.TileContext(nc) as tc:
        with (
            tc.tile_pool(name="xb", bufs=3) as xb_pool,
            tc.tile_pool(name="ys", bufs=3) as y_pool,
            tc.tile_pool(name="big", bufs=1) as big_pool,
            tc.tile_pool(name="consts", bufs=1) as const_pool,
        ):
            abig = big_pool.tile([128, T * G], fp32)       # [p,(t g)] y2
            onat = big_pool.tile([128, G * T], bf16)       # [p,(g t)] spikes
            frng = big_pool.tile([128, RING * G], fp32)    # F ring
            orng = big_pool.tile([128, RING * G], bf16)    # o ring
            dm1 = const_pool.tile([128, 1], fp32)
            ds1 = const_pool.tile([128, 1], fp32)

            a3 = abig[:].rearrange("p (t g) -> p t g", g=G)
            f3 = frng[:].rearrange("p (t g) -> p t g", g=G)
            or3 = orng[:].rearrange("p (t g) -> p t g", g=G)
            o3t = onat[:].rearrange("p (g t) -> p t g", t=T)

            loop_cm = tc.For_i(0, iters, 1) if iters > 1 else nullcontext()
            with loop_cm:
                nc.vector.memset(dm1[:], DM)
                nc.vector.memset(ds1[:], DS)
                nc.vector.memset(f3[:, 0, :], 0.0)         # F_0 = 0
                if not do_b:
                    nc.vector.memset(abig[:], 0.5)
                if not do_c:
                    nc.vector.memset(onat[:], 0.0)

                # -- phase B: DMA -> DVE scans (raw x); pure transpose-copies
                #    on ScalarE/GPSIMD into the time-major y2 buffer
                for gb in range(NB if do_b else 0):
                    xb = xb_pool.tile([128, GB * T], fp32)
                    src = x_d[gb * GB * 128:(gb + 1) * GB * 128, :]
                    nc.sync.dma_start(
                        xb[:].rearrange("p (gs t) -> p gs t", t=T),
                        src.rearrange("(gs p) t -> p gs t", p=128))
                    for gs in range(GB):
                        g = gb * GB + gs
                        y1 = y_pool.tile([128, T], fp32)
                        nc.vector.tensor_tensor_scan(
                            y1[:], dm1[:].broadcast_to([128, T]),
                            xb[:, gs * T:(gs + 1) * T],
                            0.0, Alu.mult, Alu.add)
                        nc.vector.tensor_tensor_scan(
                            y1[:], ds1[:].broadcast_to([128, T]), y1[:],
                            0.0, Alu.mult, Alu.add)
                        if g % 2 == 0:
                            nc.scalar.copy(a3[:, :, g], y1[:])
                        else:
                            nc.gpsimd.tensor_copy(a3[:, :, g], y1[:])

                # -- phase C: one fused DVE op per step; chunked recon;
                #    ScalarE+GPSIMD transpose each chunk into onat
                for c in range(NCH if do_c else 0):
                    lo, hi = c * TCH, (c + 1) * TCH
                    half = (c % NHALF) * TCH
                    for t in range(max(1, lo), hi):
                        nc.vector._custom_dve(
                            step_op,
                            out=f3[:, t % RING, :],
                            in0=f3[:, (t - 1) % RING, :],
                            in1=a3[:, t - 1, :],
                            s0=DM, imm2=GAIN)
                    nc.vector._custom_dve(
                        recon_op,
                        out=orng[:, half * G:(half + TCH) * G],
                        in0=frng[:, half * G:(half + TCH) * G],
                        in1=abig[:, lo * G:hi * G],
                        imm2=GAIN)
                    nc.scalar.copy(
                        o3t[:, lo:hi, 0:G // 2],
                        or3[:, half:half + TCH, 0:G // 2])
                    nc.gpsimd.tensor_copy(
                        o3t[:, lo:hi, G // 2:G],
                        or3[:, half:half + TCH, G // 2:G])

                # -- phase D: ONE big DMA for all spikes
                if do_d:
                    nc.sync.dma_start(
                        o_d.rearrange("(gs p) t -> p gs t", p=128),
                        onat[:].rearrange("p (gs t) -> p gs t", t=T))
                if timing:
                    nc.sync.dma_start(o_ext[:, :], onat[:, 0:T])

    nc.compile()
    return nc


def _run(x_flat: np.ndarray, iters: int = 1, trace: bool = False,
         phases: str = "full", timing: bool = False):
    from concourse.bass_utils import run_bass_kernel_spmd

    key = f"nc{iters}-{phases}-{timing}"
    if key not in _cached:
        _cached[key] = _build_program(iters, phases, timing)
    nc = _cached[key]
    if timing:
        in_maps = [{"x": np.zeros((128, T), np.float32)}
                   for _ in range(N_CORES)]
    else:
        shards = [
            np.ascontiguousarray(
                x_flat[c * ROWS_PER_CORE:(c + 1) * ROWS_PER_CORE])
            for c in range(N_CORES)
        ]
        in_maps = [{"x": s} for s in shards]
    res = run_bass_kernel_spmd(nc, in_maps, list(range(N_CORES)), trace=trace)
    outs = [np.asarray(r["o"], dtype=np.float32) for r in res.results]
    return np.concatenate(outs, axis=0), res


def kernel(x, decay_m=None, decay_s=None):
    x = np.asarray(x, dtype=np.float32)
    out_flat, _ = _run(x.reshape(ROWS, T))
    return out_flat.reshape(B, N, T)
